# revision 27
# baseline (speedup 1.0000x reference)
"""Bilateral grid slicing kernel v2.1 for Trainium2 (8 NeuronCores, SPMD).

Per pixel: z = gray(rgb); trilinear sample of (12, 8, 16, 16) grid at
(x, y, z); apply the resulting 3x4 affine to rgb.

v2.1: measured ~3.83-3.88 ms full on-device time per core (rel err
1.6e-3; Pool/gather-bound). On top of v2 (~4.26 ms measured): the cell
index is computed *before* the frac/weight ops and idxT sits in its own
3-deep pool, so tile i+1's ap_gathers (the bottleneck engine's work)
start the moment tile i's finish instead of stalling ~30us/tile on the
idx pipeline. (A dma_gather/SWDGE rewrite was tried and reverted: this
runtime's descriptor ring only fits ~65 descs/DMA, capping calls at
1024 idxs, whose fixed Pool cost erases the DMA-engine advantage —
see the session notes.)

v2 design (~125 us per 32768-pixel tile):
- Corner-baked packed table [128, 2048 cells, 3 words]: each f32 word
  packs 2 x-corner values as an fp16 pair; partition 16g+u holds combo
  u = (c4, zy) with channel c = 4*chunk + c4 and the zy=(dz,dy) corner
  offset + edge clamping baked in on the host. ONE int16 cell-index
  stream fetches all 96 corner-channel values per pixel via d=3
  ap_gathers; the gather is split into two per-tile halves that
  pipeline against compute (the software-DGE gather is the measured
  hardware bottleneck at ~50us per 2048-idx call).
- Index transpose via DMA XBAR (int16), not PE/PSUM.
- floors: q-0.5 folded into producer Act ops, then the 1.5*2^23
  float-magic round (pure f32 adds; identical on CoreSim and HW, unlike
  int casts which truncate in sim but round-to-nearest on silicon).
- Gather-space -> pixel-major via 96 PE transposes of f32 words per
  tile; packed fp16 pairs ride the f32 transpose bit-exactly (verified
  on hardware).
- Blend on DVE in fp16 at 2 elem/cycle: one mult per (chunk, half)
  whose 3D strided APs align pixel-space weights with the transposed
  gather layout (weights built in "q-order" to keep every operand
  within the TENSOR3D ISA limit), then a pairwise add-tree.
"""
import sys

sys.path.insert(0, "/opt/trn_rl_repo")
import numpy as np

import concourse.bass as bass
import concourse.bacc as bacc
import concourse.tile as tile
from concourse import mybir
from concourse import bass_utils

F32 = mybir.dt.float32
F16 = mybir.dt.float16
I16 = mybir.dt.int16
I32 = mybir.dt.int32
Alu = mybir.AluOpType
ActFn = mybir.ActivationFunctionType

N_CORES = 8
H, W = 1080, 1920
HH = H // 2                     # rows per core
P_CORE = HH * W                 # 1,036,800 pixels per core
T = 256                         # pixels per partition per tile
N_TILE = 128 * T                # 32768 pixels per tile
N_TILES = (P_CORE + N_TILE - 1) // N_TILE   # 32 (padded)
P_PAD = N_TILES * N_TILE        # 1,048,576

GL, GH, GW = 8, 16, 16
NCELL = GL * GH * GW            # 2048
FLOOR_BIAS = 0.0             # Act f32->int cast truncates toward zero == floor for q >= 0

_cache = {}


def _ap(t, extra_dims, offset=0):
    """AP on tile t keeping partition dim, custom free dims (elem units)."""
    a = t[:] if not isinstance(t, bass.AP) else t
    return bass.AP(tensor=a.tensor, offset=a.offset + offset,
                   ap=[list(a.ap[0])] + [list(d) for d in extra_dims])


def _build(n_tiles):
    nc = bacc.Bacc("TRN2", target_bir_lowering=False)
    n_pix = n_tiles * N_TILE
    with tile.TileContext(nc) as tc:
        with tc.tile_pool(name="dram", bufs=1, space="DRAM") as dram:
            gxy = dram.tile([n_pix, 2], F32, kind="ExternalInput", name="gxy", uniquify=False)
            rgb = dram.tile([n_pix, 3], F32, kind="ExternalInput", name="rgb", uniquify=False)
            tabs = dram.tile([3, 128, NCELL], F32, kind="ExternalInput", name="tabs", uniquify=False)
            ident = dram.tile([128, 128], F32, kind="ExternalInput", name="ident", uniquify=False)
            out = dram.tile([n_pix, 3], F32, kind="ExternalOutput", name="out", uniquify=False)
            _body(nc, tc, n_tiles, gxy, rgb, tabs, ident, out)
    nc.compile()
    return nc


def _build_small(n_tiles):
    """Small-loop NEFF with FULL-size io tensors: used by test.py to measure
    the on-device time differentially (equal per-call transfer cost)."""
    nc = bacc.Bacc("TRN2", target_bir_lowering=False)
    n_pix = N_TILES * N_TILE
    with tile.TileContext(nc) as tc:
        with tc.tile_pool(name="dram", bufs=1, space="DRAM") as dram:
            gxy = dram.tile([n_pix, 2], F32, kind="ExternalInput", name="gxy", uniquify=False)
            rgb = dram.tile([n_pix, 3], F32, kind="ExternalInput", name="rgb", uniquify=False)
            tabs = dram.tile([3, 128, NCELL], F32, kind="ExternalInput", name="tabs", uniquify=False)
            ident = dram.tile([128, 128], F32, kind="ExternalInput", name="ident", uniquify=False)
            out = dram.tile([n_pix, 3], F32, kind="ExternalOutput", name="out", uniquify=False)
            _body(nc, tc, n_tiles, gxy, rgb, tabs, ident, out)
    nc.compile()
    return nc


def _body(nc, tc, n_tiles, gxy, rgb, tabs, ident, out):
    import contextlib
    ctx = contextlib.ExitStack()
    const = ctx.enter_context(tc.tile_pool(name="const", bufs=1))
    io = ctx.enter_context(tc.tile_pool(name="io", bufs=3))
    wki = ctx.enter_context(tc.tile_pool(name="wki", bufs=4))
    wk = ctx.enter_context(tc.tile_pool(name="wk", bufs=2))
    gkp = ctx.enter_context(tc.tile_pool(name="gkp", bufs=3))
    mp = ctx.enter_context(tc.tile_pool(name="mp", bufs=1))
    ap_pool = ctx.enter_context(tc.tile_pool(name="ap", bufs=1))
    outp = ctx.enter_context(tc.tile_pool(name="outp", bufs=2))
    ps = ctx.enter_context(tc.tile_pool(name="ps", bufs=2, space="PSUM"))

    # --- one-time setup -------------------------------------------------
    tab_sb = const.tile([128, NCELL, 3], F32, tag="tab_sb")
    for k in range(3):
        for pb in range(0, 128, 16):   # <=32768 elems per DMA (16-bit ISA field)
            nc.sync.dma_start(out=tab_sb[pb:pb + 16, :, k], in_=tabs[k, pb:pb + 16])
    ident_sb = const.tile([128, 128], F32, tag="ident_sb")
    nc.sync.dma_start(out=ident_sb[:], in_=ident[:])

    for it in range(n_tiles):
        j0 = it * N_TILE
        gxy_t = io.tile([128, T, 2], F32, tag="gxy_t")
        nc.sync.dma_start(out=gxy_t[:], in_=gxy[j0:j0 + N_TILE, :].rearrange("(p t) c -> p t c", p=128))
        rgb_t = io.tile([128, T, 3], F32, tag="rgb_t")
        nc.sync.dma_start(out=rgb_t[:], in_=rgb[j0:j0 + N_TILE, :].rearrange("(p t) c -> p t c", p=128))

        # iz2 = 7*gray(rgb) - 0.5 (the -0.5 folded into the last addend);
        # summed on gpsimd (Pool has no scalar_tensor_tensor on HW)
        iz = wk.tile([128, T], F32, tag="iz")
        nc.scalar.activation(iz[:], rgb_t[:, :, 0], ActFn.Copy, scale=0.299 * (GL - 1))
        zt1 = wk.tile([128, T], F32, tag="zt1")
        nc.scalar.activation(zt1[:], rgb_t[:, :, 1], ActFn.Copy, scale=0.587 * (GL - 1))
        zt2 = wk.tile([128, T], F32, tag="zt2")
        nc.scalar.activation(zt2[:], rgb_t[:, :, 2], ActFn.Copy, scale=0.114 * (GL - 1), bias=-0.5)
        nc.vector.tensor_tensor(iz[:], iz[:], zt1[:], Alu.add)
        nc.vector.tensor_tensor(iz[:], iz[:], zt2[:], Alu.add)

        # --- floors: q2 = q - 0.5 (bias folded into producers), then
        # rne(q2) via the 1.5*2^23 float-magic round (pure f32 adds,
        # identical on sim and HW). frac = q2 - floor + 0.5, restored in
        # the Act ops that emit the f16 fracs. Integer ties floor to q-1
        # with frac 1.0 = the same baked +1 corner (harmless).
        ixf = wk.tile([128, T], F32, tag="ixf")
        nc.scalar.activation(ixf[:], gxy_t[:, :, 0], ActFn.Copy, scale=float(GW - 1), bias=-0.5)
        iyf = wk.tile([128, T], F32, tag="iyf")
        nc.scalar.activation(iyf[:], gxy_t[:, :, 1], ActFn.Copy, scale=float(GH - 1), bias=-0.5)

        def floor_of(q2, tag):
            qf = wk.tile([128, T], F32, tag=tag + "f")
            nc.scalar.activation(qf[:], q2[:], ActFn.Copy, bias=12582912.0)
            nc.scalar.activation(qf[:], qf[:], ActFn.Copy, bias=-12582912.0)
            return qf

        qfx = floor_of(ixf, "qx")
        qfy = floor_of(iyf, "qy")
        qfz = floor_of(iz, "qz")

        # --- flat cell index early: unblocks the gathers (the tile
        # bottleneck) before the weight ops occupy Act/DVE -------------
        idxf = wk.tile([128, T], F32, tag="idxf")
        nc.vector.scalar_tensor_tensor(idxf[:], qfz[:], float(GH), qfy[:], Alu.mult, Alu.add)
        nc.vector.scalar_tensor_tensor(idxf[:], idxf[:], float(GW), qfx[:], Alu.mult, Alu.add)
        idx16 = wk.tile([128, T], I16, tag="idx16")
        nc.scalar.activation(idx16[:], idxf[:], ActFn.Copy)
        idxT = wki.tile([128, T], I16, tag="idxT")
        for h in range(T // 128):
            nc.sync.dma_start(out=idxT[:, h * 128:(h + 1) * 128],
                              in_=idx16[:, h * 128:(h + 1) * 128], transpose=True)

        # --- fracs (f16) + 1-complements: s = q2 - floor = frac - 0.5 ---
        sx = wk.tile([128, T], F32, tag="sx")
        nc.vector.tensor_tensor(sx[:], ixf[:], qfx[:], Alu.subtract)
        sy = wk.tile([128, T], F32, tag="sy")
        nc.vector.tensor_tensor(sy[:], iyf[:], qfy[:], Alu.subtract)
        sz = wk.tile([128, T], F32, tag="sz")
        nc.vector.tensor_tensor(sz[:], iz[:], qfz[:], Alu.subtract)
        wxp = wk.tile([128, T, 2], F16, tag="wxp")      # (wx0, wx)
        nc.scalar.activation(wxp[:, :, 1], sx[:], ActFn.Copy, bias=0.5)
        nc.scalar.activation(wxp[:, :, 0], sx[:], ActFn.Copy, scale=-1.0, bias=0.5)
        wyt = wk.tile([128, T], F16, tag="wyt")
        nc.scalar.activation(wyt[:], sy[:], ActFn.Copy, bias=0.5)
        wy0 = wk.tile([128, T], F16, tag="wy0")
        nc.scalar.activation(wy0[:], sy[:], ActFn.Copy, scale=-1.0, bias=0.5)
        wzt = wk.tile([128, T], F16, tag="wzt")
        nc.scalar.activation(wzt[:], sz[:], ActFn.Copy, bias=0.5)
        wz0 = wk.tile([128, T], F16, tag="wz0")
        nc.scalar.activation(wz0[:], sz[:], ActFn.Copy, scale=-1.0, bias=0.5)

        # --- zy corner weights: v[zy] = wz_sel * wy_sel (bf16) ----------
        # v layout [128, 4, T] (zy-major blocks, contiguous T each)
        v = wk.tile([128, 4, T], F16, tag="v")
        for dz, zsel in ((0, wz0), (1, wzt)):
            for dy, ysel in ((0, wy0), (1, wyt)):
                nc.vector.tensor_tensor(v[:, dz * 2 + dy, :], zsel[:], ysel[:], Alu.mult)
        # w8 in q-order: elem = h*1024 + q*8 + zy*2 + x01 with q = rr*8 + g,
        # t = h*128 + 16g + rr. Matches the transposed-ta iteration so the
        # blend mult APs stay 3D (ISA TENSOR3D limit).
        w8 = wk.tile([128, T * 8], F16, tag="w8")
        for zy in range(4):
            for h in range(T // 128):
                nc.vector.tensor_tensor(
                    _ap(w8, [[64, 16], [8, 8], [1, 2]], offset=h * 1024 + zy * 2),
                    _ap(v, [[1, 16], [16, 8], [0, 2]], offset=zy * T + h * 128),
                    _ap(wxp, [[2, 16], [32, 8], [1, 2]], offset=h * 256),
                    Alu.mult)

        # --- A accumulation: per-half gather pipelined with compute ------
        A = ap_pool.tile([128, T * 12], F16, tag="A")   # elem = t*12 + c
        for h in range(2):
            gk = gkp.tile([128, 2048, 3], F32, tag="gk")
            nc.gpsimd.ap_gather(gk[:], tab_sb[:], idxT[:, h * 128:(h + 1) * 128],
                                channels=128, num_elems=NCELL, d=3,
                                num_idxs=2048)
            for k in range(3):
                ta = ps.tile([128, 2048], F32, tag="ta")
                for rr in range(16):
                    nc.tensor.transpose(
                        ta[:, rr * 128:(rr + 1) * 128],
                        _ap(gk, [[48, 128]], offset=rr * 3 + k),
                        ident_sb[:])
                # f16 view of ta: elem = rr*256 + g*32 + u*2 + x01
                # merged n = rr*8 + g (stride 32): 3D APs (n, c4, zyx)
                tav = ta[:].bitcast(F16)
                m = mp.tile([128, 4096], F16, tag="m")
                nc.vector.tensor_tensor(
                    _ap(m, [[32, 128], [8, 4], [1, 8]]),
                    bass.AP(tensor=tav.tensor, offset=tav.offset,
                            ap=[list(tav.ap[0]), [32, 128], [8, 4], [1, 8]]),
                    _ap(w8, [[8, 128], [0, 4], [1, 8]], offset=h * 1024),
                    Alu.mult)
                # add tree over zyx (8 -> 1) for this (chunk, half)
                s1 = mp.tile([128, 2048], F16, tag="s1")
                nc.vector.tensor_tensor(
                    _ap(s1, [[4, 512], [1, 4]]),
                    _ap(m, [[8, 512], [1, 4]]),
                    _ap(m, [[8, 512], [1, 4]], offset=4),
                    Alu.add)
                s2 = mp.tile([128, 1024], F16, tag="s2")
                nc.vector.tensor_tensor(
                    _ap(s2, [[2, 512], [1, 2]]),
                    _ap(s1, [[4, 512], [1, 2]]),
                    _ap(s1, [[4, 512], [1, 2]], offset=2),
                    Alu.add)
                # r3: A[t=h*128+16g+rr, c=4k+c4] = s2[2j] + s2[2j+1]
                nc.vector.tensor_tensor(
                    _ap(A, [[12, 16], [192, 8], [1, 4]], offset=h * 1536 + 4 * k),
                    _ap(s2, [[64, 16], [8, 8], [2, 4]]),
                    _ap(s2, [[64, 16], [8, 8], [2, 4]], offset=1),
                    Alu.add)

        # --- affine: out_i = sum_jj A[t, i*4+jj] * u4[t, jj] -------------
        rgbw = wk.tile([128, T, 4], F16, tag="rgbw")
        nc.scalar.activation(rgbw[:, :, 0:3], rgb_t[:], ActFn.Copy)
        nc.vector.memset(rgbw[:, :, 3], 1.0)
        m2 = outp.tile([128, T * 12], F16, tag="m2")    # (t, i, jj)
        nc.vector.tensor_tensor(
            _ap(m2, [[12, T], [4, 3], [1, 4]]),
            _ap(A, [[12, T], [4, 3], [1, 4]]),
            _ap(rgbw, [[4, T], [0, 3], [1, 4]]),
            Alu.mult)
        mm1 = outp.tile([128, T * 6], F16, tag="mm1")   # (t, i, jj2)
        nc.vector.tensor_tensor(
            _ap(mm1, [[6, T], [2, 3], [1, 2]]),
            _ap(m2, [[12, T], [4, 3], [1, 2]]),
            _ap(m2, [[12, T], [4, 3], [1, 2]], offset=2),
            Alu.add)
        o3 = outp.tile([128, T * 3], F16, tag="o3")     # (t, i)
        nc.vector.tensor_tensor(
            _ap(o3, [[3, T], [1, 3]]),
            _ap(mm1, [[6, T], [2, 3]]),
            _ap(mm1, [[6, T], [2, 3]], offset=1),
            Alu.add)
        outf = outp.tile([128, T * 3], F32, tag="outf")
        nc.scalar.activation(outf[:], o3[:], ActFn.Copy)
        nc.sync.dma_start(
            out=bass.AP(tensor=out.tensor, offset=out.offset + j0 * 3,
                        ap=[[T * 3, 128], [1, T * 3]]),
            in_=outf[:])
    ctx.close()


def _pack_tables(grids_view):
    """grids_view: (12, 8, 16, 16) f32 -> [3, 128, 2048] f32 packed words.

    Chunk k, partition 16g+u (replicated over g), u = c4*4 + zy with
    c = 4k + c4, zy = dz*2 + dy. Word[cell=(z,y,x)] packs fp16 pair
    (val[x], val[x+1 clamped]) of grid[c, z+dz clamped, y+dy clamped, :].
    """
    g = grids_view.astype(np.float32)  # (12, 8, 16, 16)

    def f16(a):
        return a.astype(np.float16).view(np.uint16).astype(np.uint32)

    z = np.arange(GL)[:, None, None]
    y = np.arange(GH)[None, :, None]
    x = np.arange(GW)[None, None, :]
    tabs = np.zeros((3, 128, NCELL), dtype=np.uint32)
    for k in range(3):
        for c4 in range(4):
            c = 4 * k + c4
            for dz in range(2):
                for dy in range(2):
                    u = c4 * 4 + (dz * 2 + dy)
                    zz = np.minimum(z + dz, GL - 1)
                    yy = np.minimum(y + dy, GH - 1)
                    v0 = g[c][zz, yy, x]                       # (8,16,16)
                    v1 = g[c][zz, yy, np.minimum(x + 1, GW - 1)]
                    word = f16(v0) | (f16(v1) << 16)
                    flat = word.reshape(-1)                    # z*256+y*16+x
                    for grp in range(8):
                        tabs[k, 16 * grp + u, :] = flat
    return tabs.view(np.float32)


def _shards(grid_xy, rgb, grids):
    """Split full inputs into 8 per-core input maps (padded)."""
    ident = np.eye(128, dtype=np.float32)
    maps = []
    for k in range(N_CORES):
        vv, hh = k // 2, k % 2
        gxy_s = grid_xy[vv, 0, hh * HH:(hh + 1) * HH].reshape(-1, 2)
        rgb_s = rgb[vv, 0, hh * HH:(hh + 1) * HH].reshape(-1, 3)
        pad = P_PAD - P_CORE
        gxy_s = np.concatenate([gxy_s, np.zeros((pad, 2), np.float32)])
        rgb_s = np.concatenate([rgb_s, np.zeros((pad, 3), np.float32)])
        maps.append({
            "gxy": np.ascontiguousarray(gxy_s),
            "rgb": np.ascontiguousarray(rgb_s),
            "tabs": _pack_tables(grids[vv]),
            "ident": ident,
        })
    return maps


def kernel(grid_xy, rgb, grids):
    if "nc" not in _cache:
        _cache["nc"] = _build(N_TILES)
    nc = _cache["nc"]
    maps = _shards(grid_xy, rgb, grids)
    res = bass_utils.run_bass_kernel_spmd(nc, maps, core_ids=list(range(N_CORES)))
    outv = np.empty((4, 1, H, W, 3), np.float32)
    for k in range(N_CORES):
        vv, hh = k // 2, k % 2
        o = res.results[k]["out"][:P_CORE].reshape(HH, W, 3)
        outv[vv, 0, hh * HH:(hh + 1) * HH] = o
    return outv



# revision 30
# speedup vs baseline: 1.1567x; 1.1567x over previous
"""Bilateral grid slicing kernel v2.1 for Trainium2 (8 NeuronCores, SPMD).

Per pixel: z = gray(rgb); trilinear sample of (12, 8, 16, 16) grid at
(x, y, z); apply the resulting 3x4 affine to rgb.

v2.1: measured ~3.83-3.88 ms full on-device time per core (rel err
1.6e-3; Pool/gather-bound). On top of v2 (~4.26 ms measured): the cell
index is computed *before* the frac/weight ops and idxT sits in its own
3-deep pool, so tile i+1's ap_gathers (the bottleneck engine's work)
start the moment tile i's finish instead of stalling ~30us/tile on the
idx pipeline. (A dma_gather/SWDGE rewrite was tried and reverted: this
runtime's descriptor ring only fits ~65 descs/DMA, capping calls at
1024 idxs, whose fixed Pool cost erases the DMA-engine advantage —
see the session notes.)

v2 design (~125 us per 32768-pixel tile):
- Corner-baked packed table [128, 2048 cells, 3 words]: each f32 word
  packs 2 x-corner values as an fp16 pair; partition 16g+u holds combo
  u = (c4, zy) with channel c = 4*chunk + c4 and the zy=(dz,dy) corner
  offset + edge clamping baked in on the host. ONE int16 cell-index
  stream fetches all 96 corner-channel values per pixel via d=3
  ap_gathers; the gather is split into two per-tile halves that
  pipeline against compute (the software-DGE gather is the measured
  hardware bottleneck at ~50us per 2048-idx call).
- Index transpose via DMA XBAR (int16), not PE/PSUM.
- floors: q-0.5 folded into producer Act ops, then the 1.5*2^23
  float-magic round (pure f32 adds; identical on CoreSim and HW, unlike
  int casts which truncate in sim but round-to-nearest on silicon).
- Gather-space -> pixel-major via 96 PE transposes of f32 words per
  tile; packed fp16 pairs ride the f32 transpose bit-exactly (verified
  on hardware).
- Blend on DVE in fp16 at 2 elem/cycle: one mult per (chunk, half)
  whose 3D strided APs align pixel-space weights with the transposed
  gather layout (weights built in "q-order" to keep every operand
  within the TENSOR3D ISA limit), then a pairwise add-tree.
"""
import sys

sys.path.insert(0, "/opt/trn_rl_repo")
import numpy as np

import concourse.bass as bass
import concourse.bacc as bacc
import concourse.tile as tile
from concourse import mybir
from concourse import bass_utils

F32 = mybir.dt.float32
F16 = mybir.dt.float16
I16 = mybir.dt.int16
I32 = mybir.dt.int32
Alu = mybir.AluOpType
ActFn = mybir.ActivationFunctionType

N_CORES = 8
H, W = 1080, 1920
HH = H // 2                     # rows per core
P_CORE = HH * W                 # 1,036,800 pixels per core
T = 256                         # pixels per partition per tile
N_TILE = 128 * T                # 32768 pixels per tile
N_TILES = (P_CORE + N_TILE - 1) // N_TILE   # 32 (padded)
P_PAD = N_TILES * N_TILE        # 1,048,576

GL, GH, GW = 8, 16, 16
NCELL = GL * GH * GW            # 2048
FLOOR_BIAS = 0.0             # Act f32->int cast truncates toward zero == floor for q >= 0

IO_BUFS = 3                  # input-load lookahead depth
WKI_BUFS = 4                 # idxT lookahead depth (gates the next tile's gathers)
GKP_BUFS = 3                 # gather-output buffer rotation depth

_cache = {}


def _ap(t, extra_dims, offset=0):
    """AP on tile t keeping partition dim, custom free dims (elem units)."""
    a = t[:] if not isinstance(t, bass.AP) else t
    return bass.AP(tensor=a.tensor, offset=a.offset + offset,
                   ap=[list(a.ap[0])] + [list(d) for d in extra_dims])


def _build(n_tiles):
    nc = bacc.Bacc("TRN2", target_bir_lowering=False)
    n_pix = n_tiles * N_TILE
    with tile.TileContext(nc) as tc:
        with tc.tile_pool(name="dram", bufs=1, space="DRAM") as dram:
            gxy = dram.tile([n_pix, 2], F32, kind="ExternalInput", name="gxy", uniquify=False)
            rgb = dram.tile([n_pix, 3], F32, kind="ExternalInput", name="rgb", uniquify=False)
            tabs = dram.tile([3, 128, NCELL], F32, kind="ExternalInput", name="tabs", uniquify=False)
            ident = dram.tile([128, 128], F32, kind="ExternalInput", name="ident", uniquify=False)
            out = dram.tile([n_pix, 3], F32, kind="ExternalOutput", name="out", uniquify=False)
            _body(nc, tc, n_tiles, gxy, rgb, tabs, ident, out)
    nc.compile()
    return nc


def _build_small(n_tiles):
    """Small-loop NEFF with FULL-size io tensors: used by test.py to measure
    the on-device time differentially (equal per-call transfer cost)."""
    nc = bacc.Bacc("TRN2", target_bir_lowering=False)
    n_pix = N_TILES * N_TILE
    with tile.TileContext(nc) as tc:
        with tc.tile_pool(name="dram", bufs=1, space="DRAM") as dram:
            gxy = dram.tile([n_pix, 2], F32, kind="ExternalInput", name="gxy", uniquify=False)
            rgb = dram.tile([n_pix, 3], F32, kind="ExternalInput", name="rgb", uniquify=False)
            tabs = dram.tile([3, 128, NCELL], F32, kind="ExternalInput", name="tabs", uniquify=False)
            ident = dram.tile([128, 128], F32, kind="ExternalInput", name="ident", uniquify=False)
            out = dram.tile([n_pix, 3], F32, kind="ExternalOutput", name="out", uniquify=False)
            _body(nc, tc, n_tiles, gxy, rgb, tabs, ident, out)
    nc.compile()
    return nc


def _body(nc, tc, n_tiles, gxy, rgb, tabs, ident, out):
    import contextlib
    ctx = contextlib.ExitStack()
    const = ctx.enter_context(tc.tile_pool(name="const", bufs=1))
    io = ctx.enter_context(tc.tile_pool(name="io", bufs=IO_BUFS))
    wki = ctx.enter_context(tc.tile_pool(name="wki", bufs=WKI_BUFS))
    wk = ctx.enter_context(tc.tile_pool(name="wk", bufs=2))
    gkp = ctx.enter_context(tc.tile_pool(name="gkp", bufs=GKP_BUFS))
    mp = ctx.enter_context(tc.tile_pool(name="mp", bufs=1))
    ap_pool = ctx.enter_context(tc.tile_pool(name="ap", bufs=1))
    outp = ctx.enter_context(tc.tile_pool(name="outp", bufs=2))
    ps = ctx.enter_context(tc.tile_pool(name="ps", bufs=2, space="PSUM"))

    # --- one-time setup -------------------------------------------------
    tab_sb = const.tile([128, NCELL, 3], F32, tag="tab_sb")
    for k in range(3):
        for pb in range(0, 128, 16):   # <=32768 elems per DMA (16-bit ISA field)
            nc.sync.dma_start(out=tab_sb[pb:pb + 16, :, k], in_=tabs[k, pb:pb + 16])
    ident_sb = const.tile([128, 128], F32, tag="ident_sb")
    nc.sync.dma_start(out=ident_sb[:], in_=ident[:])

    for it in range(n_tiles):
        j0 = it * N_TILE
        gxy_t = io.tile([128, T, 2], F32, tag="gxy_t")
        nc.sync.dma_start(out=gxy_t[:], in_=gxy[j0:j0 + N_TILE, :].rearrange("(p t) c -> p t c", p=128))
        rgb_t = io.tile([128, T, 3], F32, tag="rgb_t")
        nc.sync.dma_start(out=rgb_t[:], in_=rgb[j0:j0 + N_TILE, :].rearrange("(p t) c -> p t c", p=128))

        # iz2 = 7*gray(rgb) - 0.5 (the -0.5 folded into the last addend);
        # summed on gpsimd (Pool has no scalar_tensor_tensor on HW)
        iz = wk.tile([128, T], F32, tag="iz")
        nc.scalar.activation(iz[:], rgb_t[:, :, 0], ActFn.Copy, scale=0.299 * (GL - 1))
        zt1 = wk.tile([128, T], F32, tag="zt1")
        nc.scalar.activation(zt1[:], rgb_t[:, :, 1], ActFn.Copy, scale=0.587 * (GL - 1))
        zt2 = wk.tile([128, T], F32, tag="zt2")
        nc.scalar.activation(zt2[:], rgb_t[:, :, 2], ActFn.Copy, scale=0.114 * (GL - 1), bias=-0.5)
        nc.vector.tensor_tensor(iz[:], iz[:], zt1[:], Alu.add)
        nc.vector.tensor_tensor(iz[:], iz[:], zt2[:], Alu.add)

        # --- floors: q2 = q - 0.5 (bias folded into producers), then
        # rne(q2) via the 1.5*2^23 float-magic round (pure f32 adds,
        # identical on sim and HW). frac = q2 - floor + 0.5, restored in
        # the Act ops that emit the f16 fracs. Integer ties floor to q-1
        # with frac 1.0 = the same baked +1 corner (harmless).
        ixf = wk.tile([128, T], F32, tag="ixf")
        nc.scalar.activation(ixf[:], gxy_t[:, :, 0], ActFn.Copy, scale=float(GW - 1), bias=-0.5)
        iyf = wk.tile([128, T], F32, tag="iyf")
        nc.scalar.activation(iyf[:], gxy_t[:, :, 1], ActFn.Copy, scale=float(GH - 1), bias=-0.5)

        def floor_of(q2, tag):
            qf = wk.tile([128, T], F32, tag=tag + "f")
            nc.scalar.activation(qf[:], q2[:], ActFn.Copy, bias=12582912.0)
            nc.scalar.activation(qf[:], qf[:], ActFn.Copy, bias=-12582912.0)
            return qf

        qfx = floor_of(ixf, "qx")
        qfy = floor_of(iyf, "qy")
        qfz = floor_of(iz, "qz")

        # --- flat cell index early: unblocks the gathers (the tile
        # bottleneck) before the weight ops occupy Act/DVE -------------
        idxf = wk.tile([128, T], F32, tag="idxf")
        nc.vector.scalar_tensor_tensor(idxf[:], qfz[:], float(GH), qfy[:], Alu.mult, Alu.add)
        nc.vector.scalar_tensor_tensor(idxf[:], idxf[:], float(GW), qfx[:], Alu.mult, Alu.add)
        idx16 = wk.tile([128, T], I16, tag="idx16")
        nc.scalar.activation(idx16[:], idxf[:], ActFn.Copy)
        idxT = wki.tile([128, T], I16, tag="idxT")
        for h in range(T // 128):
            nc.sync.dma_start(out=idxT[:, h * 128:(h + 1) * 128],
                              in_=idx16[:, h * 128:(h + 1) * 128], transpose=True)

        # --- fracs (f16) + 1-complements: s = q2 - floor = frac - 0.5 ---
        sx = wk.tile([128, T], F32, tag="sx")
        nc.vector.tensor_tensor(sx[:], ixf[:], qfx[:], Alu.subtract)
        sy = wk.tile([128, T], F32, tag="sy")
        nc.vector.tensor_tensor(sy[:], iyf[:], qfy[:], Alu.subtract)
        sz = wk.tile([128, T], F32, tag="sz")
        nc.vector.tensor_tensor(sz[:], iz[:], qfz[:], Alu.subtract)
        wxp = wk.tile([128, T, 2], F16, tag="wxp")      # (wx0, wx)
        nc.scalar.activation(wxp[:, :, 1], sx[:], ActFn.Copy, bias=0.5)
        nc.scalar.activation(wxp[:, :, 0], sx[:], ActFn.Copy, scale=-1.0, bias=0.5)
        wyt = wk.tile([128, T], F16, tag="wyt")
        nc.scalar.activation(wyt[:], sy[:], ActFn.Copy, bias=0.5)
        wy0 = wk.tile([128, T], F16, tag="wy0")
        nc.scalar.activation(wy0[:], sy[:], ActFn.Copy, scale=-1.0, bias=0.5)
        wzt = wk.tile([128, T], F16, tag="wzt")
        nc.scalar.activation(wzt[:], sz[:], ActFn.Copy, bias=0.5)
        wz0 = wk.tile([128, T], F16, tag="wz0")
        nc.scalar.activation(wz0[:], sz[:], ActFn.Copy, scale=-1.0, bias=0.5)

        # --- zy corner weights: v[zy] = wz_sel * wy_sel (bf16) ----------
        # v layout [128, 4, T] (zy-major blocks, contiguous T each)
        v = wk.tile([128, 4, T], F16, tag="v")
        for dz, zsel in ((0, wz0), (1, wzt)):
            for dy, ysel in ((0, wy0), (1, wyt)):
                nc.vector.tensor_tensor(v[:, dz * 2 + dy, :], zsel[:], ysel[:], Alu.mult)
        # w8 in q-order: elem = h*1024 + q*8 + zy*2 + x01 with q = rr*8 + g,
        # t = h*128 + 16g + rr. Matches the transposed-ta iteration so the
        # blend mult APs stay 3D (ISA TENSOR3D limit).
        w8 = wk.tile([128, T * 8], F16, tag="w8")
        for zy in range(4):
            for h in range(T // 128):
                nc.vector.tensor_tensor(
                    _ap(w8, [[64, 16], [8, 8], [1, 2]], offset=h * 1024 + zy * 2),
                    _ap(v, [[1, 16], [16, 8], [0, 2]], offset=zy * T + h * 128),
                    _ap(wxp, [[2, 16], [32, 8], [1, 2]], offset=h * 256),
                    Alu.mult)

        # --- A accumulation: per-half gather pipelined with compute ------
        A = ap_pool.tile([128, T * 12], F16, tag="A")   # elem = t*12 + c
        for h in range(2):
            gk = gkp.tile([128, 2048, 3], F32, tag="gk")
            nc.gpsimd.ap_gather(gk[:], tab_sb[:], idxT[:, h * 128:(h + 1) * 128],
                                channels=128, num_elems=NCELL, d=3,
                                num_idxs=2048)
            for k in range(3):
                ta = ps.tile([128, 2048], F32, tag="ta")
                for rr in range(16):
                    nc.tensor.transpose(
                        ta[:, rr * 128:(rr + 1) * 128],
                        _ap(gk, [[48, 128]], offset=rr * 3 + k),
                        ident_sb[:])
                # f16 view of ta: elem = rr*256 + g*32 + u*2 + x01
                # merged n = rr*8 + g (stride 32): 3D APs (n, c4, zyx)
                tav = ta[:].bitcast(F16)
                m = mp.tile([128, 4096], F16, tag="m")
                nc.vector.tensor_tensor(
                    _ap(m, [[32, 128], [8, 4], [1, 8]]),
                    bass.AP(tensor=tav.tensor, offset=tav.offset,
                            ap=[list(tav.ap[0]), [32, 128], [8, 4], [1, 8]]),
                    _ap(w8, [[8, 128], [0, 4], [1, 8]], offset=h * 1024),
                    Alu.mult)
                # add tree over zyx (8 -> 1) for this (chunk, half)
                s1 = mp.tile([128, 2048], F16, tag="s1")
                nc.vector.tensor_tensor(
                    _ap(s1, [[4, 512], [1, 4]]),
                    _ap(m, [[8, 512], [1, 4]]),
                    _ap(m, [[8, 512], [1, 4]], offset=4),
                    Alu.add)
                s2 = mp.tile([128, 1024], F16, tag="s2")
                nc.vector.tensor_tensor(
                    _ap(s2, [[2, 512], [1, 2]]),
                    _ap(s1, [[4, 512], [1, 2]]),
                    _ap(s1, [[4, 512], [1, 2]], offset=2),
                    Alu.add)
                # r3: A[t=h*128+16g+rr, c=4k+c4] = s2[2j] + s2[2j+1]
                nc.vector.tensor_tensor(
                    _ap(A, [[12, 16], [192, 8], [1, 4]], offset=h * 1536 + 4 * k),
                    _ap(s2, [[64, 16], [8, 8], [2, 4]]),
                    _ap(s2, [[64, 16], [8, 8], [2, 4]], offset=1),
                    Alu.add)

        # --- affine: out_i = sum_jj A[t, i*4+jj] * u4[t, jj] -------------
        rgbw = wk.tile([128, T, 4], F16, tag="rgbw")
        nc.scalar.activation(rgbw[:, :, 0:3], rgb_t[:], ActFn.Copy)
        nc.vector.memset(rgbw[:, :, 3], 1.0)
        m2 = outp.tile([128, T * 12], F16, tag="m2")    # (t, i, jj)
        nc.vector.tensor_tensor(
            _ap(m2, [[12, T], [4, 3], [1, 4]]),
            _ap(A, [[12, T], [4, 3], [1, 4]]),
            _ap(rgbw, [[4, T], [0, 3], [1, 4]]),
            Alu.mult)
        mm1 = outp.tile([128, T * 6], F16, tag="mm1")   # (t, i, jj2)
        nc.vector.tensor_tensor(
            _ap(mm1, [[6, T], [2, 3], [1, 2]]),
            _ap(m2, [[12, T], [4, 3], [1, 2]]),
            _ap(m2, [[12, T], [4, 3], [1, 2]], offset=2),
            Alu.add)
        o3 = outp.tile([128, T * 3], F16, tag="o3")     # (t, i)
        nc.vector.tensor_tensor(
            _ap(o3, [[3, T], [1, 3]]),
            _ap(mm1, [[6, T], [2, 3]]),
            _ap(mm1, [[6, T], [2, 3]], offset=1),
            Alu.add)
        outf = outp.tile([128, T * 3], F32, tag="outf")
        nc.scalar.activation(outf[:], o3[:], ActFn.Copy)
        nc.sync.dma_start(
            out=bass.AP(tensor=out.tensor, offset=out.offset + j0 * 3,
                        ap=[[T * 3, 128], [1, T * 3]]),
            in_=outf[:])
    ctx.close()


def _pack_tables(grids_view):
    """grids_view: (12, 8, 16, 16) f32 -> [3, 128, 2048] f32 packed words.

    Chunk k, partition 16g+u (replicated over g), u = c4*4 + zy with
    c = 4k + c4, zy = dz*2 + dy. Word[cell=(z,y,x)] packs fp16 pair
    (val[x], val[x+1 clamped]) of grid[c, z+dz clamped, y+dy clamped, :].
    """
    g = grids_view.astype(np.float32)  # (12, 8, 16, 16)

    def f16(a):
        return a.astype(np.float16).view(np.uint16).astype(np.uint32)

    z = np.arange(GL)[:, None, None]
    y = np.arange(GH)[None, :, None]
    x = np.arange(GW)[None, None, :]
    tabs = np.zeros((3, 128, NCELL), dtype=np.uint32)
    for k in range(3):
        for c4 in range(4):
            c = 4 * k + c4
            for dz in range(2):
                for dy in range(2):
                    u = c4 * 4 + (dz * 2 + dy)
                    zz = np.minimum(z + dz, GL - 1)
                    yy = np.minimum(y + dy, GH - 1)
                    v0 = g[c][zz, yy, x]                       # (8,16,16)
                    v1 = g[c][zz, yy, np.minimum(x + 1, GW - 1)]
                    word = f16(v0) | (f16(v1) << 16)
                    flat = word.reshape(-1)                    # z*256+y*16+x
                    for grp in range(8):
                        tabs[k, 16 * grp + u, :] = flat
    return tabs.view(np.float32)


def _shards(grid_xy, rgb, grids):
    """Split full inputs into 8 per-core input maps (padded)."""
    ident = np.eye(128, dtype=np.float32)
    maps = []
    for k in range(N_CORES):
        vv, hh = k // 2, k % 2
        gxy_s = grid_xy[vv, 0, hh * HH:(hh + 1) * HH].reshape(-1, 2)
        rgb_s = rgb[vv, 0, hh * HH:(hh + 1) * HH].reshape(-1, 3)
        pad = P_PAD - P_CORE
        gxy_s = np.concatenate([gxy_s, np.zeros((pad, 2), np.float32)])
        rgb_s = np.concatenate([rgb_s, np.zeros((pad, 3), np.float32)])
        maps.append({
            "gxy": np.ascontiguousarray(gxy_s),
            "rgb": np.ascontiguousarray(rgb_s),
            "tabs": _pack_tables(grids[vv]),
            "ident": ident,
        })
    return maps


def kernel(grid_xy, rgb, grids):
    if "nc" not in _cache:
        _cache["nc"] = _build(N_TILES)
    nc = _cache["nc"]
    maps = _shards(grid_xy, rgb, grids)
    res = bass_utils.run_bass_kernel_spmd(nc, maps, core_ids=list(range(N_CORES)))
    outv = np.empty((4, 1, H, W, 3), np.float32)
    for k in range(N_CORES):
        vv, hh = k // 2, k % 2
        o = res.results[k]["out"][:P_CORE].reshape(HH, W, 3)
        outv[vv, 0, hh * HH:(hh + 1) * HH] = o
    return outv



# revision 31
# speedup vs baseline: 1.2231x; 1.0574x over previous
"""Bilateral grid slicing kernel v2.1 for Trainium2 (8 NeuronCores, SPMD).

Per pixel: z = gray(rgb); trilinear sample of (12, 8, 16, 16) grid at
(x, y, z); apply the resulting 3x4 affine to rgb.

v2.1: measured ~3.83-3.88 ms full on-device time per core (rel err
1.6e-3; Pool/gather-bound). On top of v2 (~4.26 ms measured): the cell
index is computed *before* the frac/weight ops and idxT sits in its own
3-deep pool, so tile i+1's ap_gathers (the bottleneck engine's work)
start the moment tile i's finish instead of stalling ~30us/tile on the
idx pipeline. (A dma_gather/SWDGE rewrite was tried and reverted: this
runtime's descriptor ring only fits ~65 descs/DMA, capping calls at
1024 idxs, whose fixed Pool cost erases the DMA-engine advantage —
see the session notes.)

v2 design (~125 us per 32768-pixel tile):
- Corner-baked packed table [128, 2048 cells, 3 words]: each f32 word
  packs 2 x-corner values as an fp16 pair; partition 16g+u holds combo
  u = (c4, zy) with channel c = 4*chunk + c4 and the zy=(dz,dy) corner
  offset + edge clamping baked in on the host. ONE int16 cell-index
  stream fetches all 96 corner-channel values per pixel via d=3
  ap_gathers; the gather is split into two per-tile halves that
  pipeline against compute (the software-DGE gather is the measured
  hardware bottleneck at ~50us per 2048-idx call).
- Index transpose via DMA XBAR (int16), not PE/PSUM.
- floors: q-0.5 folded into producer Act ops, then the 1.5*2^23
  float-magic round (pure f32 adds; identical on CoreSim and HW, unlike
  int casts which truncate in sim but round-to-nearest on silicon).
- Gather-space -> pixel-major via 96 PE transposes of f32 words per
  tile; packed fp16 pairs ride the f32 transpose bit-exactly (verified
  on hardware).
- Blend on DVE in fp16 at 2 elem/cycle: one mult per (chunk, half)
  whose 3D strided APs align pixel-space weights with the transposed
  gather layout (weights built in "q-order" to keep every operand
  within the TENSOR3D ISA limit), then a pairwise add-tree.
"""
import sys

sys.path.insert(0, "/opt/trn_rl_repo")
import numpy as np

import concourse.bass as bass
import concourse.bacc as bacc
import concourse.tile as tile
from concourse import mybir
from concourse import bass_utils

F32 = mybir.dt.float32
F16 = mybir.dt.float16
I16 = mybir.dt.int16
I32 = mybir.dt.int32
Alu = mybir.AluOpType
ActFn = mybir.ActivationFunctionType

N_CORES = 8
H, W = 1080, 1920
HH = H // 2                     # rows per core
P_CORE = HH * W                 # 1,036,800 pixels per core
T = 256                         # pixels per partition per tile
N_TILE = 128 * T                # 32768 pixels per tile
N_TILES = (P_CORE + N_TILE - 1) // N_TILE   # 32 (padded)
P_PAD = N_TILES * N_TILE        # 1,048,576

GL, GH, GW = 8, 16, 16
NCELL = GL * GH * GW            # 2048
FLOOR_BIAS = 0.0             # Act f32->int cast truncates toward zero == floor for q >= 0

IO_BUFS = 3                  # input-load lookahead depth
WKI_BUFS = 4                 # idxT lookahead depth (gates the next tile's gathers)
GKP_BUFS = 3                 # gather-output buffer rotation depth
SKIP_BLEND = False           # diagnostic: drop transposes+blend (A = memset)

_cache = {}


def _ap(t, extra_dims, offset=0):
    """AP on tile t keeping partition dim, custom free dims (elem units)."""
    a = t[:] if not isinstance(t, bass.AP) else t
    return bass.AP(tensor=a.tensor, offset=a.offset + offset,
                   ap=[list(a.ap[0])] + [list(d) for d in extra_dims])


def _build(n_tiles):
    nc = bacc.Bacc("TRN2", target_bir_lowering=False)
    n_pix = n_tiles * N_TILE
    with tile.TileContext(nc) as tc:
        with tc.tile_pool(name="dram", bufs=1, space="DRAM") as dram:
            gxy = dram.tile([n_pix, 2], F32, kind="ExternalInput", name="gxy", uniquify=False)
            rgb = dram.tile([n_pix, 3], F32, kind="ExternalInput", name="rgb", uniquify=False)
            tabs = dram.tile([3, 128, NCELL], F32, kind="ExternalInput", name="tabs", uniquify=False)
            ident = dram.tile([128, 128], F32, kind="ExternalInput", name="ident", uniquify=False)
            out = dram.tile([n_pix, 3], F32, kind="ExternalOutput", name="out", uniquify=False)
            _body(nc, tc, n_tiles, gxy, rgb, tabs, ident, out)
    nc.compile()
    return nc


def _build_small(n_tiles):
    """Small-loop NEFF with FULL-size io tensors: used by test.py to measure
    the on-device time differentially (equal per-call transfer cost)."""
    nc = bacc.Bacc("TRN2", target_bir_lowering=False)
    n_pix = N_TILES * N_TILE
    with tile.TileContext(nc) as tc:
        with tc.tile_pool(name="dram", bufs=1, space="DRAM") as dram:
            gxy = dram.tile([n_pix, 2], F32, kind="ExternalInput", name="gxy", uniquify=False)
            rgb = dram.tile([n_pix, 3], F32, kind="ExternalInput", name="rgb", uniquify=False)
            tabs = dram.tile([3, 128, NCELL], F32, kind="ExternalInput", name="tabs", uniquify=False)
            ident = dram.tile([128, 128], F32, kind="ExternalInput", name="ident", uniquify=False)
            out = dram.tile([n_pix, 3], F32, kind="ExternalOutput", name="out", uniquify=False)
            _body(nc, tc, n_tiles, gxy, rgb, tabs, ident, out)
    nc.compile()
    return nc


def _body(nc, tc, n_tiles, gxy, rgb, tabs, ident, out):
    import contextlib
    ctx = contextlib.ExitStack()
    const = ctx.enter_context(tc.tile_pool(name="const", bufs=1))
    io = ctx.enter_context(tc.tile_pool(name="io", bufs=IO_BUFS))
    wki = ctx.enter_context(tc.tile_pool(name="wki", bufs=WKI_BUFS))
    wk = ctx.enter_context(tc.tile_pool(name="wk", bufs=2))
    gkp = ctx.enter_context(tc.tile_pool(name="gkp", bufs=GKP_BUFS))
    mp = ctx.enter_context(tc.tile_pool(name="mp", bufs=1))
    ap_pool = ctx.enter_context(tc.tile_pool(name="ap", bufs=1))
    outp = ctx.enter_context(tc.tile_pool(name="outp", bufs=2))
    ps = ctx.enter_context(tc.tile_pool(name="ps", bufs=2, space="PSUM"))

    # --- one-time setup -------------------------------------------------
    tab_sb = const.tile([128, NCELL, 3], F32, tag="tab_sb")
    for k in range(3):
        for pb in range(0, 128, 16):   # <=32768 elems per DMA (16-bit ISA field)
            nc.sync.dma_start(out=tab_sb[pb:pb + 16, :, k], in_=tabs[k, pb:pb + 16])
    ident_sb = const.tile([128, 128], F32, tag="ident_sb")
    nc.sync.dma_start(out=ident_sb[:], in_=ident[:])

    for it in range(n_tiles):
        j0 = it * N_TILE
        gxy_t = io.tile([128, T, 2], F32, tag="gxy_t")
        nc.sync.dma_start(out=gxy_t[:], in_=gxy[j0:j0 + N_TILE, :].rearrange("(p t) c -> p t c", p=128))
        rgb_t = io.tile([128, T, 3], F32, tag="rgb_t")
        nc.sync.dma_start(out=rgb_t[:], in_=rgb[j0:j0 + N_TILE, :].rearrange("(p t) c -> p t c", p=128))

        # iz2 = 7*gray(rgb) - 0.5 (the -0.5 folded into the last addend);
        # summed on gpsimd (Pool has no scalar_tensor_tensor on HW)
        iz = wk.tile([128, T], F32, tag="iz")
        nc.scalar.activation(iz[:], rgb_t[:, :, 0], ActFn.Copy, scale=0.299 * (GL - 1))
        zt1 = wk.tile([128, T], F32, tag="zt1")
        nc.scalar.activation(zt1[:], rgb_t[:, :, 1], ActFn.Copy, scale=0.587 * (GL - 1))
        zt2 = wk.tile([128, T], F32, tag="zt2")
        nc.scalar.activation(zt2[:], rgb_t[:, :, 2], ActFn.Copy, scale=0.114 * (GL - 1), bias=-0.5)
        nc.vector.tensor_tensor(iz[:], iz[:], zt1[:], Alu.add)
        nc.vector.tensor_tensor(iz[:], iz[:], zt2[:], Alu.add)

        # --- floors: q2 = q - 0.5 (bias folded into producers), then
        # rne(q2) via the 1.5*2^23 float-magic round (pure f32 adds,
        # identical on sim and HW). frac = q2 - floor + 0.5, restored in
        # the Act ops that emit the f16 fracs. Integer ties floor to q-1
        # with frac 1.0 = the same baked +1 corner (harmless).
        ixf = wk.tile([128, T], F32, tag="ixf")
        nc.scalar.activation(ixf[:], gxy_t[:, :, 0], ActFn.Copy, scale=float(GW - 1), bias=-0.5)
        iyf = wk.tile([128, T], F32, tag="iyf")
        nc.scalar.activation(iyf[:], gxy_t[:, :, 1], ActFn.Copy, scale=float(GH - 1), bias=-0.5)

        def floor_of(q2, tag):
            qf = wk.tile([128, T], F32, tag=tag + "f")
            nc.scalar.activation(qf[:], q2[:], ActFn.Copy, bias=12582912.0)
            nc.scalar.activation(qf[:], qf[:], ActFn.Copy, bias=-12582912.0)
            return qf

        qfx = floor_of(ixf, "qx")
        qfy = floor_of(iyf, "qy")
        qfz = floor_of(iz, "qz")

        # --- flat cell index early: unblocks the gathers (the tile
        # bottleneck) before the weight ops occupy Act/DVE -------------
        idxf = wk.tile([128, T], F32, tag="idxf")
        nc.vector.scalar_tensor_tensor(idxf[:], qfz[:], float(GH), qfy[:], Alu.mult, Alu.add)
        nc.vector.scalar_tensor_tensor(idxf[:], idxf[:], float(GW), qfx[:], Alu.mult, Alu.add)
        idx16 = wk.tile([128, T], I16, tag="idx16")
        nc.scalar.activation(idx16[:], idxf[:], ActFn.Copy)
        idxT = wki.tile([128, T], I16, tag="idxT")
        for h in range(T // 128):
            nc.sync.dma_start(out=idxT[:, h * 128:(h + 1) * 128],
                              in_=idx16[:, h * 128:(h + 1) * 128], transpose=True)

        # --- fracs (f16) + 1-complements: s = q2 - floor = frac - 0.5 ---
        sx = wk.tile([128, T], F32, tag="sx")
        nc.vector.tensor_tensor(sx[:], ixf[:], qfx[:], Alu.subtract)
        sy = wk.tile([128, T], F32, tag="sy")
        nc.vector.tensor_tensor(sy[:], iyf[:], qfy[:], Alu.subtract)
        sz = wk.tile([128, T], F32, tag="sz")
        nc.vector.tensor_tensor(sz[:], iz[:], qfz[:], Alu.subtract)
        wxp = wk.tile([128, T, 2], F16, tag="wxp")      # (wx0, wx)
        nc.scalar.activation(wxp[:, :, 1], sx[:], ActFn.Copy, bias=0.5)
        nc.scalar.activation(wxp[:, :, 0], sx[:], ActFn.Copy, scale=-1.0, bias=0.5)
        wyt = wk.tile([128, T], F16, tag="wyt")
        nc.scalar.activation(wyt[:], sy[:], ActFn.Copy, bias=0.5)
        wy0 = wk.tile([128, T], F16, tag="wy0")
        nc.scalar.activation(wy0[:], sy[:], ActFn.Copy, scale=-1.0, bias=0.5)
        wzt = wk.tile([128, T], F16, tag="wzt")
        nc.scalar.activation(wzt[:], sz[:], ActFn.Copy, bias=0.5)
        wz0 = wk.tile([128, T], F16, tag="wz0")
        nc.scalar.activation(wz0[:], sz[:], ActFn.Copy, scale=-1.0, bias=0.5)

        # --- zy corner weights: v[zy] = wz_sel * wy_sel (bf16) ----------
        # v layout [128, 4, T] (zy-major blocks, contiguous T each)
        v = wk.tile([128, 4, T], F16, tag="v")
        for dz, zsel in ((0, wz0), (1, wzt)):
            for dy, ysel in ((0, wy0), (1, wyt)):
                nc.vector.tensor_tensor(v[:, dz * 2 + dy, :], zsel[:], ysel[:], Alu.mult)
        # w8 in q-order: elem = h*1024 + q*8 + zy*2 + x01 with q = rr*8 + g,
        # t = h*128 + 16g + rr. Matches the transposed-ta iteration so the
        # blend mult APs stay 3D (ISA TENSOR3D limit).
        w8 = wk.tile([128, T * 8], F16, tag="w8")
        for zy in range(4):
            for h in range(T // 128):
                nc.vector.tensor_tensor(
                    _ap(w8, [[64, 16], [8, 8], [1, 2]], offset=h * 1024 + zy * 2),
                    _ap(v, [[1, 16], [16, 8], [0, 2]], offset=zy * T + h * 128),
                    _ap(wxp, [[2, 16], [32, 8], [1, 2]], offset=h * 256),
                    Alu.mult)

        # --- A accumulation: per-half gather pipelined with compute ------
        A = ap_pool.tile([128, T * 12], F16, tag="A")   # elem = t*12 + c
        for h in range(2):
            gk = gkp.tile([128, 2048, 3], F32, tag="gk")
            nc.gpsimd.ap_gather(gk[:], tab_sb[:], idxT[:, h * 128:(h + 1) * 128],
                                channels=128, num_elems=NCELL, d=3,
                                num_idxs=2048)
            if SKIP_BLEND:
                continue
            for k in range(3):
                ta = ps.tile([128, 2048], F32, tag="ta")
                for rr in range(16):
                    nc.tensor.transpose(
                        ta[:, rr * 128:(rr + 1) * 128],
                        _ap(gk, [[48, 128]], offset=rr * 3 + k),
                        ident_sb[:])
                # f16 view of ta: elem = rr*256 + g*32 + u*2 + x01
                # merged n = rr*8 + g (stride 32): 3D APs (n, c4, zyx)
                tav = ta[:].bitcast(F16)
                m = mp.tile([128, 4096], F16, tag="m")
                nc.vector.tensor_tensor(
                    _ap(m, [[32, 128], [8, 4], [1, 8]]),
                    bass.AP(tensor=tav.tensor, offset=tav.offset,
                            ap=[list(tav.ap[0]), [32, 128], [8, 4], [1, 8]]),
                    _ap(w8, [[8, 128], [0, 4], [1, 8]], offset=h * 1024),
                    Alu.mult)
                # add tree over zyx (8 -> 1) for this (chunk, half)
                s1 = mp.tile([128, 2048], F16, tag="s1")
                nc.vector.tensor_tensor(
                    _ap(s1, [[4, 512], [1, 4]]),
                    _ap(m, [[8, 512], [1, 4]]),
                    _ap(m, [[8, 512], [1, 4]], offset=4),
                    Alu.add)
                s2 = mp.tile([128, 1024], F16, tag="s2")
                nc.vector.tensor_tensor(
                    _ap(s2, [[2, 512], [1, 2]]),
                    _ap(s1, [[4, 512], [1, 2]]),
                    _ap(s1, [[4, 512], [1, 2]], offset=2),
                    Alu.add)
                # r3: A[t=h*128+16g+rr, c=4k+c4] = s2[2j] + s2[2j+1]
                nc.vector.tensor_tensor(
                    _ap(A, [[12, 16], [192, 8], [1, 4]], offset=h * 1536 + 4 * k),
                    _ap(s2, [[64, 16], [8, 8], [2, 4]]),
                    _ap(s2, [[64, 16], [8, 8], [2, 4]], offset=1),
                    Alu.add)

        # --- affine: out_i = sum_jj A[t, i*4+jj] * u4[t, jj] -------------
        if SKIP_BLEND:
            nc.vector.memset(A[:], 0.0)
        rgbw = wk.tile([128, T, 4], F16, tag="rgbw")
        nc.scalar.activation(rgbw[:, :, 0:3], rgb_t[:], ActFn.Copy)
        nc.vector.memset(rgbw[:, :, 3], 1.0)
        m2 = outp.tile([128, T * 12], F16, tag="m2")    # (t, i, jj)
        nc.vector.tensor_tensor(
            _ap(m2, [[12, T], [4, 3], [1, 4]]),
            _ap(A, [[12, T], [4, 3], [1, 4]]),
            _ap(rgbw, [[4, T], [0, 3], [1, 4]]),
            Alu.mult)
        mm1 = outp.tile([128, T * 6], F16, tag="mm1")   # (t, i, jj2)
        nc.vector.tensor_tensor(
            _ap(mm1, [[6, T], [2, 3], [1, 2]]),
            _ap(m2, [[12, T], [4, 3], [1, 2]]),
            _ap(m2, [[12, T], [4, 3], [1, 2]], offset=2),
            Alu.add)
        o3 = outp.tile([128, T * 3], F16, tag="o3")     # (t, i)
        nc.vector.tensor_tensor(
            _ap(o3, [[3, T], [1, 3]]),
            _ap(mm1, [[6, T], [2, 3]]),
            _ap(mm1, [[6, T], [2, 3]], offset=1),
            Alu.add)
        outf = outp.tile([128, T * 3], F32, tag="outf")
        nc.scalar.activation(outf[:], o3[:], ActFn.Copy)
        nc.sync.dma_start(
            out=bass.AP(tensor=out.tensor, offset=out.offset + j0 * 3,
                        ap=[[T * 3, 128], [1, T * 3]]),
            in_=outf[:])
    ctx.close()


def _pack_tables(grids_view):
    """grids_view: (12, 8, 16, 16) f32 -> [3, 128, 2048] f32 packed words.

    Chunk k, partition 16g+u (replicated over g), u = c4*4 + zy with
    c = 4k + c4, zy = dz*2 + dy. Word[cell=(z,y,x)] packs fp16 pair
    (val[x], val[x+1 clamped]) of grid[c, z+dz clamped, y+dy clamped, :].
    """
    g = grids_view.astype(np.float32)  # (12, 8, 16, 16)

    def f16(a):
        return a.astype(np.float16).view(np.uint16).astype(np.uint32)

    z = np.arange(GL)[:, None, None]
    y = np.arange(GH)[None, :, None]
    x = np.arange(GW)[None, None, :]
    tabs = np.zeros((3, 128, NCELL), dtype=np.uint32)
    for k in range(3):
        for c4 in range(4):
            c = 4 * k + c4
            for dz in range(2):
                for dy in range(2):
                    u = c4 * 4 + (dz * 2 + dy)
                    zz = np.minimum(z + dz, GL - 1)
                    yy = np.minimum(y + dy, GH - 1)
                    v0 = g[c][zz, yy, x]                       # (8,16,16)
                    v1 = g[c][zz, yy, np.minimum(x + 1, GW - 1)]
                    word = f16(v0) | (f16(v1) << 16)
                    flat = word.reshape(-1)                    # z*256+y*16+x
                    for grp in range(8):
                        tabs[k, 16 * grp + u, :] = flat
    return tabs.view(np.float32)


def _shards(grid_xy, rgb, grids):
    """Split full inputs into 8 per-core input maps (padded)."""
    ident = np.eye(128, dtype=np.float32)
    maps = []
    for k in range(N_CORES):
        vv, hh = k // 2, k % 2
        gxy_s = grid_xy[vv, 0, hh * HH:(hh + 1) * HH].reshape(-1, 2)
        rgb_s = rgb[vv, 0, hh * HH:(hh + 1) * HH].reshape(-1, 3)
        pad = P_PAD - P_CORE
        gxy_s = np.concatenate([gxy_s, np.zeros((pad, 2), np.float32)])
        rgb_s = np.concatenate([rgb_s, np.zeros((pad, 3), np.float32)])
        maps.append({
            "gxy": np.ascontiguousarray(gxy_s),
            "rgb": np.ascontiguousarray(rgb_s),
            "tabs": _pack_tables(grids[vv]),
            "ident": ident,
        })
    return maps


def kernel(grid_xy, rgb, grids):
    if "nc" not in _cache:
        _cache["nc"] = _build(N_TILES)
    nc = _cache["nc"]
    maps = _shards(grid_xy, rgb, grids)
    res = bass_utils.run_bass_kernel_spmd(nc, maps, core_ids=list(range(N_CORES)))
    outv = np.empty((4, 1, H, W, 3), np.float32)
    for k in range(N_CORES):
        vv, hh = k // 2, k % 2
        o = res.results[k]["out"][:P_CORE].reshape(HH, W, 3)
        outv[vv, 0, hh * HH:(hh + 1) * HH] = o
    return outv

